# revision 21
# baseline (speedup 1.0000x reference)
"""Trainium2 Bass kernel for linear attention (silu+1 feature map, cumsum over T)
with dense 1024x1024 in/out projections.

Sharding: 8 cores = 4 batches x 2 head-groups (8 heads / 512 channels each).
Each core computes q/k/v projections for its 512 channels over the full
T=4096 of its batch, the linear-attention recurrence locally (DVE prefix
scan along T), and a partial Wo projection (512 in-ch -> all 1024 out-ch).
The host sums the two partials per batch and adds bo. No cross-core traffic.

Layout: channels on partitions, tokens on the free dim. Channels are
head-INTERLEAVED within each 128-partition chunk (partition p of chunk c
holds head 2c + p%2) so the 1/den broadcast can be done by a DMA with a
0-stride source (DRAM bounce) instead of a PE matmul.

vs the previous version (202us -> target ~140us):
 - rb (per-head 1/den broadcast to 128 partitions) moved off the PE onto
   the DMA engines via a DRAM bounce: rec[8,SL] -> dram -> rb[128,SL]
   with a 64x replicated read. Frees 2 PSUM banks and ~9us of PE time.
 - PSUM: q/k projection tiles get 2 bufs each (recycle slack), v 1,
   den 1, wo 2 = 8 banks. Kills the ACT-recycle PE stalls.
 - pq/nm via stock scalar_tensor_tensor (drops the custom AFFINE op).
 - rec stays fp32 (reciprocal_approx_fast), no bf16 copy; at reads rb fp32.
 - HAM warmup: a few dummy DR matmuls on zeroed scratch run during the
   DMA head so real matmuls start at 2.4GHz instead of 1.2GHz.
 - head DMAs split across two queues (weights+bounce on Sync, x+out on
   GpSimd), consts first, so the first real matmul starts ~5us earlier.
 - per-slab emission interleaves tail work (den/recip/bounce mid-slab,
   nm/at at slab end, wo/ot of slab s-2 spread across chunk slots) so no
   engine FIFO head-of-line-blocks another.
"""

import numpy as np
import ml_dtypes

import concourse.bass as bass
import concourse.mybir as mybir
from concourse import bacc, tile
from concourse.bass_utils import run_bass_kernel_spmd

BF16 = mybir.dt.bfloat16
F32 = mybir.dt.float32
FP8 = mybir.dt.float8e4
DR = mybir.MatmulPerfMode.DoubleRow
XS = 0.125        # host scales x by XS, weights by 1/(XS*PS)
PS = 0.125        # ACT scale undoing the fp8 pre-scaling: psum*PS = true value
ADD = mybir.AluOpType.add
MULT = mybir.AluOpType.mult
SILU = mybir.ActivationFunctionType.Silu
COPY = mybir.ActivationFunctionType.Copy
IDENT = mybir.ActivationFunctionType.Identity

B, C, T = 4, 1024, 4096
H, DH = 16, 64
CG = 512            # channels per head-group (per core)
S = 512             # token slab
NCH = CG // 128     # 4 chunks of 128 channels
KCH = C // 128      # 8 input-channel chunks
MO = C // 128       # 8 output-channel chunks
N_WARM = 11         # HAM warmup matmuls

SLABS = []
_off = 0
for _sz in [512] * 8:
    SLABS.append((_off, _sz))
    _off += _sz


def register_scan_ops():
    """Register full-rate custom DVE scan ops (idempotent)."""
    import concourse.dve_ops as dops
    from concourse.dve_spec import Spec, Src0, Src1, C0, Bin, AluOp, One, scan, lower
    from concourse.dve_uop import DveOpSpec

    existing = {o.name: o for o in dops.OPS}
    if "SCAN_KS_ANT" in existing:
        return existing["SCAN_KS_ANT"], existing["SCAN_KVS_ANT"]

    defs = [
        ("SCAN_KS_ANT",
         Spec(body=scan(AluOp.ADD, Bin(AluOp.ADD, Src0, One), init=C0),
              reference=lambda in0, in1, s0, s1, imm2:
                  np.cumsum(in0.astype(np.float32) + 1.0, axis=1) + s0),
         False),
        ("SCAN_KVS_ANT",
         Spec(body=scan(AluOp.ADD,
                        Bin(AluOp.MULTIPLY, Bin(AluOp.ADD, Src0, One), Src1),
                        init=C0),
              reference=lambda in0, in1, s0, s1, imm2:
                  np.cumsum((in0.astype(np.float32) + 1.0)
                            * in1.astype(np.float32), axis=1) + s0),
         True),
    ]
    out = []
    for name, spec, rd1 in defs:
        row = dops._CUSTOM_DVE_ROW_BASE + len(dops.OPS)
        shas = {}
        for ver in ("v3", "v4"):
            try:
                s = DveOpSpec(name=name, opcode=row, uops=lower(spec, ver=ver), rd1_en=rd1)
                shas[ver] = s.sha(ver)
            except Exception:
                pass
        op = dops.DveOp(name, spec, subdim=False, uops_sha=shas)
        dops.OPS.append(op)
        dops.CUSTOM_DVE_SPECS[name] = spec
        dops._SUB_OPCODE_FOR_NAME[name] = row
        out.append(op)
    return out[0], out[1]


def build():
    """Build the per-core Bass program (identical on all 8 cores)."""
    KS_OP, KVS_OP = register_scan_ops()
    nc = bacc.Bacc(target_bir_lowering=False)

    # x and the weights are pre-interleaved on the host into the DoubleRow
    # (ki, ko) layout so every DMA descriptor covers >=1KB contiguous.
    x_d = nc.declare_dram_parameter("x", [8 * 128, T], FP8, isOutput=False)
    wq_d = nc.declare_dram_parameter("wq", [C // 2, 2 * CG], FP8, isOutput=False)
    wk_d = nc.declare_dram_parameter("wk", [C // 2, 2 * CG], FP8, isOutput=False)
    wv_d = nc.declare_dram_parameter("wv", [C // 2, 2 * CG], FP8, isOutput=False)
    wo_d = nc.declare_dram_parameter("wo", [CG // 2, 2 * C], FP8, isOutput=False)
    cb_d = nc.declare_dram_parameter("consts", [128, 8], F32, isOutput=False)
    em_d = nc.declare_dram_parameter("emat", [128, 32], BF16, isOutput=False)
    fm_d = nc.declare_dram_parameter("fmat", [8, CG], BF16, isOutput=False)
    out_d = nc.declare_dram_parameter("out", [C, T], BF16, isOutput=True)

    # DRAM bounce buffer for the 1/den broadcast (per-slab column slices)
    rbb_d = nc.dram_tensor("rbounce", [8, T], F32, kind="Internal")

    with tile.TileContext(nc) as tc:
        from contextlib import ExitStack

        with ExitStack() as ctx:
            wpool = ctx.enter_context(tc.tile_pool(name="w", bufs=1))
            xpool = ctx.enter_context(tc.tile_pool(name="xp", bufs=3))
            qpool = ctx.enter_context(tc.tile_pool(name="qp", bufs=2, space="PSUM"))
            kpool = ctx.enter_context(tc.tile_pool(name="kp", bufs=2, space="PSUM"))
            vpool = ctx.enter_context(tc.tile_pool(name="vp", bufs=1, space="PSUM"))
            dpool = ctx.enter_context(tc.tile_pool(name="denp", bufs=1, space="PSUM"))
            opool = ctx.enter_context(tc.tile_pool(name="wops", bufs=2, space="PSUM"))
            apool = ctx.enter_context(tc.tile_pool(name="act", bufs=2))
            spool = ctx.enter_context(tc.tile_pool(name="state", bufs=2))
            rpool = ctx.enter_context(tc.tile_pool(name="rb", bufs=2))
            outpool = ctx.enter_context(tc.tile_pool(name="outp", bufs=2))

            def load(pool, shape, dtype, src, tag, q=None):
                t = pool.tile(shape, dtype, tag=tag, name=tag)
                (q or nc.sync).dma_start(t[:], src)
                return t

            # ---- HAM warmup: dummy DR matmuls on zeroed scratch so the PE
            # clock ramps to 2.4GHz while the first DMAs are in flight.
            warm_w = wpool.tile([128, 2, 128], FP8, tag="warmw", name="warmw")
            warm_x = wpool.tile([128, 2, 512], FP8, tag="warmx", name="warmx")
            nc.vector.memset(warm_w[:], 0.0)
            nc.vector.memset(warm_x[:], 0.0)
            warm_ps = qpool.tile([128, 512], F32, tag="q", name="warm_ps")
            for i in range(N_WARM):
                nc.tensor.matmul(warm_ps[:], warm_w[:], warm_x[:],
                                 start=True, stop=True, perf_mode=DR)

            # ---- weights on the Sync queue (wq first -- it gates the first
            # real matmul), consts next, x on the GpSimd queue in parallel.
            def load_w8(dram, tagp, n, fd):
                tiles = []
                for k in range(n):
                    t = wpool.tile([128, 2, fd], FP8, tag=f"{tagp}{k}", name=f"{tagp}{k}")
                    nc.sync.dma_start(t[:], dram[128 * k: 128 * (k + 1), :].rearrange("p (ko m) -> p ko m", ko=2))
                    tiles.append(t)
                return tiles

            wq_t = load_w8(wq_d, "wq", KCH // 2, CG)
            cb_t = load(wpool, [128, 8], F32, cb_d[:, :], "consts")
            em2_t = load(wpool, [128, 32], BF16, em_d[:, :], "em")
            fm_t = load(wpool, [8, CG], BF16, fm_d[:, :], "fm")
            bq_t = [cb_t[:, c: c + 1] for c in range(NCH)]
            bv_t = [cb_t[:, 4 + c: 5 + c] for c in range(NCH)]
            em_t = [em2_t[:, 8 * c: 8 * (c + 1)] for c in range(NCH)]

            def load_x(s, t0, SL):
                sb512, off = t0 // 512, t0 % 512
                xt = xpool.tile([128, KCH // 2, 2, SL], FP8, tag="x", name=f"x_{s}")
                xsrc = x_d[128 * sb512: 128 * (sb512 + 1), :]
                xsrc = xsrc.rearrange("p (c ko t) -> p c ko t", c=4, ko=2)[:, :, :, off: off + SL]
                q = nc.gpsimd if s == 0 else nc.sync
                q.dma_start(xt[:], xsrc)
                return xt

            x_t = load_x(0, 0, 512)
            wk_t = load_w8(wk_d, "wk", KCH // 2, CG)
            wv_t = load_w8(wv_d, "wv", KCH // 2, CG)
            wo_t = load_w8(wo_d, "wo", NCH // 2, C)

            K2 = KCH // 2

            def proj(ps, w_t, x_t, cs, SL):
                for k in range(K2):
                    nc.tensor.matmul(ps[:], w_t[k][:, :, cs], x_t[:, k, :, :],
                                     start=(k == 0), stop=(k == K2 - 1), perf_mode=DR)

            # per-slab state
            prev_ks = [None] * NCH
            prev_kvs = [None] * NCH
            prev_len = None
            # deferred work descriptors
            pend_a = None       # (s, t0, SL, sq_l, ks_l, kvs_l, pq_l) -> den/recip/bounce
            pend_wo = None      # (s, t0, SL, at_l) -> wo/ot/outDMA
            pend_ot = None      # deferred ot for the last wo slot of a slab
            prev_slot = None

            def emit_q(s, SL, c, x_t, sq_l):
                cs = slice(128 * c, 128 * (c + 1))
                ps_q = qpool.tile([128, SL], F32, tag="q", name=f"psq{s}_{c}")
                proj(ps_q, wq_t, x_t, cs, SL)
                sq = apool.tile([128, SL], BF16, tag=f"sq{c}", name=f"sq{s}_{c}")
                nc.scalar.activation(sq[:], ps_q[:], SILU, bias=bq_t[c], scale=PS)
                sq_l.append(sq)

            def emit_kv(s, SL, c, x_t, sq_l, ks_l, kvs_l, pq_l):
                cs = slice(128 * c, 128 * (c + 1))
                ps_v = vpool.tile([128, SL], F32, tag="v", name=f"psv{s}_{c}")
                proj(ps_v, wv_t, x_t, cs, SL)
                vs = apool.tile([128, SL], BF16, tag=f"vs{c}", name=f"vs{s}_{c}")
                nc.scalar.activation(vs[:], ps_v[:], IDENT, bias=bv_t[c], scale=PS)
                ps_k = kpool.tile([128, SL], F32, tag="k", name=f"psk{s}_{c}")
                proj(ps_k, wk_t, x_t, cs, SL)
                sk = apool.tile([128, SL], BF16, tag=f"sk{c}", name=f"sk{s}_{c}")
                nc.scalar.activation(sk[:], ps_k[:], SILU, scale=PS)

                ks = spool.tile([128, SL], F32, tag=f"ks{c}", name=f"ks{s}_{c}")
                ik = 0.0 if s == 0 else prev_ks[c][:, prev_len - 1: prev_len]
                nc.vector._custom_dve(KS_OP, out=ks[:], in0=sk[:], s0=ik)
                pq = apool.tile([128, SL], BF16, tag=f"pq{c}", name=f"pq{s}_{c}")
                nc.vector.scalar_tensor_tensor(pq[:], sq_l[c][:], 1.0, ks[:], ADD, MULT)
                kvs = spool.tile([128, SL], F32, tag=f"kvs{c}", name=f"kvs{s}_{c}")
                ikv = 0.0 if s == 0 else prev_kvs[c][:, prev_len - 1: prev_len]
                nc.vector._custom_dve(KVS_OP, out=kvs[:], in0=sk[:], in1=vs[:], s0=ikv)
                prev_ks[c], prev_kvs[c] = ks, kvs

                ks_l.append(ks), kvs_l.append(kvs), pq_l.append(pq)

            def emit_den_bounce(s, t0, SL, pq_l):
                """den matmuls + reciprocal + DMA bounce broadcast for slab s."""
                den_ps = dpool.tile([8, SL], F32, tag="den", name=f"den{s}")
                for c in range(NCH):
                    nc.tensor.matmul(den_ps[:], em_t[c], pq_l[c][:],
                                     start=(c == 0), stop=(c == NCH - 1))
                rec = spool.tile([8, SL], F32, tag="rec", name=f"rec{s}")
                nc.vector.reciprocal_approx_fast(rec[:], den_ps[:])
                nc.sync.dma_start(rbb_d[:, t0: t0 + SL], rec[:])
                rb_l = []
                for c in range(NCH):
                    rb = rpool.tile([128, SL], F32, tag=f"rb{c}", name=f"rb{s}_{c}")
                    nc.sync.dma_start(
                        rb[:], rbb_d[2 * c: 2 * c + 2, t0: t0 + SL].partition_broadcast(64))
                    rb_l.append(rb)
                return rb_l

            def alloc_at(s, SL):
                return [outpool.tile([128, 2, SL], FP8, tag=f"at{cc}", name=f"at{s}_{cc}")
                        for cc in range(NCH // 2)]

            def emit_nm_at(s, SL, sq_l, kvs_l, rb_l, at_l, chunks):
                for c in chunks:
                    nm = apool.tile([128, SL], BF16, tag=f"nm{c}", name=f"nm{s}_{c}")
                    nc.vector.scalar_tensor_tensor(nm[:], sq_l[c][:], 1.0, kvs_l[c][:], ADD, MULT)
                    nc.vector.tensor_mul(at_l[c // 2][:, c % 2, :], nm[:], rb_l[c][:])
                return at_l

            def emit_wo_mms(s, t0, SL, at_l, moo):
                """wo matmuls for output group moo (2 mo chunks)."""
                ps_pair = []
                for mo2 in range(2):
                    mo = 2 * moo + mo2
                    ms = slice(128 * mo, 128 * (mo + 1))
                    wo_ps = opool.tile([128, SL], F32, tag="wo", name=f"wo{s}_{mo}")
                    for kk in range(NCH // 2):
                        nc.tensor.matmul(wo_ps[:], wo_t[kk][:, :, ms], at_l[kk][:],
                                         start=(kk == 0), stop=(kk == NCH // 2 - 1), perf_mode=DR)
                    ps_pair.append(wo_ps)
                return (s, t0, SL, ps_pair, moo)

            def emit_ot(s, t0, SL, ps_pair, moo):
                """PSUM->SBUF conversion + out-DMA for a wo pair (deferred one
                chunk slot so waiting ot ACTs never block projection ACTs)."""
                tts = slice(t0, t0 + SL)
                ot = outpool.tile([128, 2, SL], BF16, tag=f"ot{moo}", name=f"ot{s}_{moo}")
                for mo2 in range(2):
                    nc.scalar.activation(ot[:, mo2, :], ps_pair[mo2][:], COPY, scale=1.0 / 2048.0)
                nc.gpsimd.dma_start(
                    out_d[256 * moo: 256 * (moo + 1), tts].rearrange("(mo2 ki) t -> ki mo2 t", mo2=2),
                    ot[:])

            def emit_wo_slot(s, t0, SL, at_l, moo):
                emit_ot(*emit_wo_mms(s, t0, SL, at_l, moo))

            for s, (t0, SL) in enumerate(SLABS):
                if s > 0:
                    x_t = load_x(s, t0, SL)
                sq_l, ks_l, kvs_l, pq_l = [], [], [], []

                if s == 0:
                    # special head order: all q chunks first (only wq+x are in
                    # flight at this point), then v/k + scans per chunk.
                    for c in range(NCH):
                        emit_q(s, SL, c, x_t, sq_l)
                    for c in range(NCH):
                        emit_kv(s, SL, c, x_t, sq_l, ks_l, kvs_l, pq_l)
                else:
                    rb_l = None
                    at_l = None
                    pa_prev = pend_a
                    for c in range(NCH):
                        emit_q(s, SL, c, x_t, sq_l)
                        emit_kv(s, SL, c, x_t, sq_l, ks_l, kvs_l, pq_l)
                        if c == 0:
                            rb_l = emit_den_bounce(pa_prev[0], pa_prev[1],
                                                   pa_prev[2], pa_prev[6])
                        if pend_wo is not None and c >= 1:
                            pw = pend_wo
                            emit_wo_slot(pw[0], pw[1], pw[2], pw[3], c - 1)

                    # end of slab: nm/at for s-1 (rb now in flight/ready)
                    at_l = alloc_at(pa_prev[0], pa_prev[2])
                    emit_nm_at(pa_prev[0], pa_prev[2], pa_prev[3],
                               pa_prev[5], rb_l, at_l, (0, 1, 2, 3))
                    if pend_wo is not None:
                        pw = pend_wo
                        emit_wo_slot(pw[0], pw[1], pw[2], pw[3], 3)
                    pend_wo = (pa_prev[0], pa_prev[1], pa_prev[2], at_l)
                pend_a = (s, t0, SL, sq_l, ks_l, kvs_l, pq_l)
                prev_len = SL

            # ---- drain: final slab uses PE-based rb (q/k PSUM banks are
            # free now) to skip the ~6us DRAM bounce latency on the critical
            # tail; wo(7) runs on the PE while the DVE chews nm/at(8).
            pa = pend_a
            SLf = pa[2]

            _ka = [0]

            def keepalive(n):
                # fresh q-tag tiles each batch: proper WAR tracking on the
                # recycled PSUM slot (writing the long-dead warm_ps would race)
                _ka[0] += 1
                kt = qpool.tile([128, 512], F32, tag="q", name=f"ka{_ka[0]}")
                for _ in range(n):
                    nc.tensor.matmul(kt[:], warm_w[:], warm_x[:],
                                     start=True, stop=True, perf_mode=DR)

            keepalive(6)
            den_ps = dpool.tile([8, SLf], F32, tag="den", name="den_f")
            for c in range(NCH):
                nc.tensor.matmul(den_ps[:], em_t[c], pa[6][c][:],
                                 start=(c == 0), stop=(c == NCH - 1))
            rec = spool.tile([8, SLf], F32, tag="rec", name="rec_f")
            nc.vector.reciprocal_approx_fast(rec[:], den_ps[:])
            rec_bf = spool.tile([8, SLf], BF16, tag="recbf", name="rec_bf")
            nc.vector.tensor_copy(rec_bf[:], rec[:])

            # wo(s-2) keeps the PE busy while the DVE drains nm/at chains
            keepalive(4)
            if pend_wo is not None:
                pw = pend_wo
                for moo in range(4):
                    emit_wo_slot(pw[0], pw[1], pw[2], pw[3], moo)
            rb_l = []
            for c in range(NCH):
                pool_f = qpool if c < 2 else kpool
                rb = pool_f.tile([128, SLf], F32, tag=("q" if c < 2 else "k"),
                                 name=f"rbf_{c}")
                nc.tensor.matmul(rb[:], fm_t[:, 128 * c: 128 * (c + 1)], rec_bf[:],
                                 start=True, stop=True)
                rb_l.append(rb)
            at_l = alloc_at(pa[0], pa[2])
            emit_nm_at(pa[0], pa[2], pa[3], pa[5], rb_l, at_l, (0, 1, 2, 3))
            keepalive(12)
            for moo in range(4):
                emit_wo_slot(pa[0], pa[1], pa[2], at_l, moo)
                keepalive(2)

    nc.compile()
    return nc


_NC_CACHE = {}


def _get_nc():
    if "nc" not in _NC_CACHE:
        _NC_CACHE["nc"] = build()
    return _NC_CACHE["nc"]


# tile-position -> canonical channel permutation (head-interleaved chunks):
# position j = c*128 + p holds canonical channel (2c + p%2)*64 + p//2
PERM = np.empty(CG, np.int64)
for _c in range(NCH):
    for _p in range(128):
        PERM[_c * 128 + _p] = (2 * _c + _p % 2) * 64 + _p // 2


def make_in_maps(x, Wq, bq, Wk, Wv, bv, Wo, bo):
    bf = ml_dtypes.bfloat16
    f8 = ml_dtypes.float8_e4m3
    WS = 1.0 / (XS * PS)  # weight pre-scale so that psum * PS = W @ x exactly
    x3 = np.asarray(x, np.float32)[..., 0]                      # (B, C, T)
    E = np.zeros((CG, 8), np.float32)
    for j in range(CG):
        E[j, (2 * (j // 128)) + (j % 128) % 2] = 1.0
    # em packed per chunk: [128, 4*8]; fm = transpose for the PE rb fallback
    EM = np.zeros((128, 32), np.float32)
    for c in range(4):
        EM[:, 8 * c: 8 * (c + 1)] = E[128 * c: 128 * (c + 1), :] / 32.0
    FM = np.ascontiguousarray(E.T)  # [8, 512] tile-ordered

    def dr_w(arr):
        # (Cin, M) -> [Cin/2 partitions, 2*M] with the DoubleRow (ki, ko)
        # interleave baked in so the SBUF load is a contiguous copy
        cin, m = arr.shape
        return np.ascontiguousarray(
            arr.reshape(cin // 256, 2, 128, m).transpose(0, 2, 1, 3).reshape(cin // 2, 2 * m))

    def dr_x(arr):
        # (C, T) -> [8*128, 4096]: row = slab*128 + partition, cols =
        # (cgroup, ko, t512) so one DMA with 4KB-contiguous runs loads a slab
        return np.ascontiguousarray(
            arr.reshape(4, 2, 128, 8, 512).transpose(3, 2, 0, 1, 4).reshape(1024, 4096))

    in_maps = []
    for core in range(8):
        b, g = core // 2, core % 2
        sl = slice(CG * g, CG * (g + 1))
        Wq_c = np.asarray(Wq, np.float32)[sl, :][PERM, :]   # (512 out, 1024 in)
        Wk_c = np.asarray(Wk, np.float32)[sl, :][PERM, :]
        Wv_c = np.asarray(Wv, np.float32)[sl, :][PERM, :]
        Wo_c = np.asarray(Wo, np.float32)[:, sl][:, PERM]   # (1024 out, 512 in)
        in_maps.append({
            "x": dr_x(np.clip(x3[b] * XS, -240, 240).astype(f8)),
            "wq": dr_w(np.clip(np.ascontiguousarray(Wq_c.T) * WS, -240, 240).astype(f8)),
            "wk": dr_w(np.clip(np.ascontiguousarray(Wk_c.T) * WS, -240, 240).astype(f8)),
            "wv": dr_w(np.clip(np.ascontiguousarray(Wv_c.T) * WS, -240, 240).astype(f8)),
            "wo": dr_w(np.clip(np.ascontiguousarray((Wo_c * 0.125).T) * 64.0, -240, 240).astype(f8)),
            "consts": np.concatenate([
                np.asarray(bq, np.float32)[sl][PERM].reshape(4, 128).T,
                np.asarray(bv, np.float32)[sl][PERM].reshape(4, 128).T], axis=1).copy(),
            "emat": EM.astype(bf),
            "fmat": FM.astype(bf),
        })
    return in_maps


def assemble(results, bo):
    out = np.empty((B, C, T, 1), np.float32)
    bo_f = np.asarray(bo, np.float32)[:, None]
    for b in range(B):
        p0 = np.asarray(results[2 * b]["out"], np.float32)
        p1 = np.asarray(results[2 * b + 1]["out"], np.float32)
        out[b, :, :, 0] = p0 + p1 + bo_f
    return out


def kernel(x, Wq, bq, Wk, Wv, bv, Wo, bo):
    nc = _get_nc()
    in_maps = make_in_maps(x, Wq, bq, Wk, Wv, bv, Wo, bo)
    res = run_bass_kernel_spmd(nc, in_maps, core_ids=list(range(8)))
    return assemble(res.results, bo)
